# revision 16
# baseline (speedup 1.0000x reference)
"""Trainium2 Bass kernel for nn_BasicBlock — windowed dma_gather + scatter-add design.

Computation (reference):
    h1 = sum_k mask1[k,n] * x[kmap1[k,n]] @ W1[k]
    o1 = relu(bn(h1))
    h2 = sum_k mask2[k,n] * o1[kmap2[k,n]] @ W2[k]
    out = relu(bn(h2) + x)

Voxels sharded 8 ways (25000/core); the feature table (x for conv1, the
all-gathered normalized o1 for conv2) is replicated in DRAM as bf16.

The old per-128-row indirect-DMA gather costs ~1.4us of Pool-sequencer time
per call (measured), an ~15ms floor for 1.35M rows. Instead, each conv is
reorganized around InstDMAGatherAnt/InstDMAScatterAddAnt, whose int16 index
limit is met by splitting the 200256-row table into 7 windows of 32768 rows:

  for each window w, k-triple: one dma_gather(transpose=True) fetches ALL
  live slots (mask=1) whose source row falls in w — compact, int16
  window-local indices, output already transposed [ci, slots];
  per k: W_k^T @ compact -> PSUM; PE-transposes back to row-major;
  one dma_scatter_add adds the rows into h-slab[k%4] at their voxel ids
  (int16, < 25088). Masked slots are never gathered (halves traffic).

h = sum of 4 slabs, computed during a streaming stats pass (transposed into
SBUF); BN stats all-reduced; bn1+relu rows all-gathered (bf16) for conv2;
bn2 + f32 residual + relu written per-core and concatenated on the host.
Index lists are padded to static shapes (gather pad -> window row 0,
scatter pad -> -1 which the ucode ignores), so the program is input-shape
independent; counts are asserted on the host.
"""
import math
from contextlib import ExitStack

import numpy as np

N_GLOB = 200000
C = 128
K = 27
N_CORES = 8
EPS = 1e-5

R = N_GLOB // N_CORES          # 25000 valid rows per core
TILES = math.ceil(R / 128)     # 196
R_PAD = TILES * 128            # 25088
SUP = 4
NSUP = math.ceil(TILES / SUP)  # 49
TABLE_ROWS = ((N_GLOB + 128 + 127) // 128) * 128 + 64  # 200256
ZROW = N_GLOB

WROWS = 28672                  # gather window rows (int16-addressable)
NW = 7                         # uniform windows (last covers 28224 table rows)
S = 2048                       # padded slots per (w, k); gathers issued in 512-chunks
GCH = 512                      # max idx per dma_gather/scatter call (hw ring limit)
NSLAB = 4                      # h accumulation slabs ((2k+chunk) % 8)
SC_DELAY = 4                   # scatter issued this many (w,k) iters late
SC_CH = 512                    # idxs per scatter call; keep == GCH so every
                               # gather/scatter shares one num_idxs register
                               # (mixed sizes raced on the sequencer GPR)

_TRACE = False
_TMPDIR = None
LAST_RESULTS = None

_NC_CACHE = {}


def _build():
    from concourse import bass, bacc, tile, mybir

    f32 = mybir.dt.float32
    bf16 = mybir.dt.bfloat16
    i16 = mybir.dt.int16
    AF = mybir.ActivationFunctionType
    ALU = mybir.AluOpType
    AX = mybir.AxisListType

    r_pad = R_PAD
    nsup = NSUP
    rg = [list(range(N_CORES))]

    nc = bacc.Bacc("TRN2", target_bir_lowering=False, debug=False,
                   num_devices=N_CORES, num_swdge_queues=4)

    x_aug = nc.dram_tensor("x_aug", [TABLE_ROWS, C], bf16, kind="ExternalInput").ap()
    # idx planes: [w, k, 128, S/16] (wrap16 layout)
    g1b = nc.dram_tensor("g1b", [NW, K, 128, S // 16], i16, kind="ExternalInput").ap()
    s1b = nc.dram_tensor("s1b", [NW, K, 128, S // 16], i16, kind="ExternalInput").ap()
    g2b = nc.dram_tensor("g2b", [NW, K, 128, S // 16], i16, kind="ExternalInput").ap()
    s2b = nc.dram_tensor("s2b", [NW, K, 128, S // 16], i16, kind="ExternalInput").ap()
    w1d = nc.dram_tensor("w1", [K, C, C], bf16, kind="ExternalInput").ap()
    w2d = nc.dram_tensor("w2", [K, C, C], bf16, kind="ExternalInput").ap()
    bnt = nc.dram_tensor("bnt", [C, 4], f32, kind="ExternalInput").ap()
    xres = nc.dram_tensor("xres", [r_pad, C], f32, kind="ExternalInput").ap()
    y = nc.dram_tensor("y", [r_pad, C], f32, kind="ExternalOutput").ap()

    with tile.TileContext(nc) as tc, ExitStack() as ctx:
        dram = ctx.enter_context(tc.tile_pool(name="dram", bufs=1, space="DRAM"))
        cc_in = dram.tile([R, C], bf16, tag="cc_in")
        cc_out = dram.tile([TABLE_ROWS, C], bf16, tag="cc_out")
        ccs_in = dram.tile([C, 2], f32, tag="ccs_in")
        ccs_out = dram.tile([C, 2], f32, tag="ccs_out")
        ccs2_in = dram.tile([C, 2], f32, tag="ccs2_in")
        ccs2_out = dram.tile([C, 2], f32, tag="ccs2_out")
        slabs1 = [dram.tile([r_pad + 128, C], bf16, tag=f"hs1_{i}", name=f"hs1_{i}")
                  for i in range(NSLAB)]
        slabs2 = [dram.tile([r_pad + 128, C], bf16, tag=f"hs2_{i}", name=f"hs2_{i}")
                  for i in range(NSLAB)]

        wpool = ctx.enter_context(tc.tile_pool(name="wpool", bufs=1))
        hpool = ctx.enter_context(tc.tile_pool(name="hpool", bufs=1))
        spool = ctx.enter_context(tc.tile_pool(name="spool", bufs=1))
        gipool = ctx.enter_context(tc.tile_pool(name="gipool", bufs=12))
        sipool = ctx.enter_context(tc.tile_pool(name="sipool", bufs=12))
        gcpool = ctx.enter_context(tc.tile_pool(name="gcpool", bufs=10))
        sdpool = ctx.enter_context(tc.tile_pool(name="sdpool", bufs=10))
        gtpool = ctx.enter_context(tc.tile_pool(name="gtpool", bufs=6))
        rpool = ctx.enter_context(tc.tile_pool(name="rpool", bufs=2))
        opool = ctx.enter_context(tc.tile_pool(name="opool", bufs=2))
        pmp = ctx.enter_context(tc.tile_pool(name="pmp", bufs=3, space="PSUM"))
        ptp = ctx.enter_context(tc.tile_pool(name="ptp", bufs=4, space="PSUM"))

        # --- resident constants ---
        from concourse.masks import make_identity
        identf = wpool.tile([128, 128], f32, tag="identf")
        make_identity(nc, identf[:])
        ident = wpool.tile([128, 128], bf16, tag="ident")
        nc.vector.tensor_copy(out=ident[:], in_=identf[:])

        w1s = wpool.tile([128, K * C], bf16, tag="w1")
        w2s = wpool.tile([128, K * C], bf16, tag="w2")
        nc.sync.dma_start(out=w1s[:].rearrange("p (k c) -> p k c", c=C),
                          in_=w1d.rearrange("k ci co -> ci k co"))
        nc.sync.dma_start(out=w2s[:].rearrange("p (k c) -> p k c", c=C),
                          in_=w2d.rearrange("k ci co -> ci k co"))
        bns = wpool.tile([128, 4], f32, tag="bns")
        nc.sync.dma_start(out=bns[:], in_=bnt[:, :])

        zt = wpool.tile([128, 8 * C], bf16, tag="zt")
        nc.vector.memset(zt[:], 0.0)

        def zero_slabs(slabs):
            for sl in slabs:
                off = 0
                while off < r_pad:
                    n = min(1024, r_pad - off)
                    nc.sync.dma_start(
                        out=sl[off:off + n, :].rearrange("(b p) c -> p b c", p=128),
                        in_=zt[:].rearrange("p (b c) -> p b c", c=C)[:, :n // 128, :])
                    off += n

        # zero the tail rows of cc_out (mask-skipped rows never read; keep clean)
        off = N_GLOB
        while off < TABLE_ROWS:
            n = min(1024, TABLE_ROWS - off)
            nc.sync.dma_start(
                out=cc_out[off:off + n, :].rearrange("(b p) c -> p b c", p=128),
                in_=zt[:].rearrange("p (b c) -> p b c", c=C)[:, :n // 128, :])
            off += n

        st_sum1 = spool.tile([128, nsup], f32, tag="st_sum1")
        st_sq1 = spool.tile([128, nsup], f32, tag="st_sq1")
        st_sum2 = spool.tile([128, nsup], f32, tag="st_sum2")
        st_sq2 = spool.tile([128, nsup], f32, tag="st_sq2")

        def conv(src_ap, gb, sb, w_sb, slabs):
            # Software-pipelined: the scatter for iteration i is issued D
            # iterations later, so the in-order Pool queue never stalls on
            # the gather->matmul->transpose->copy chain of the current
            # iteration. One 2048-idx scatter per (w,k) (measured 2.7ns/row,
            # same per-row rate as 512-idx calls at 1/4 the Pool calls).
            pend = []

            def flush_one():
                sit_, sd_, k_ = pend.pop(0)
                nch = S // SC_CH
                for g in range(nch):
                    nc.gpsimd.dma_scatter_add(
                        out_ap=slabs[k_ % NSLAB][:, :],
                        in_ap=sd_[:, g * SC_CH:(g + 1) * SC_CH].rearrange(
                            "p (b c) -> p b c", c=C),
                        idxs_ap=sit_[:, g * SC_CH // 16:(g + 1) * SC_CH // 16],
                        num_idxs=SC_CH, num_idxs_reg=SC_CH,
                        elem_size=C, queue_num=k_ % 4)

            for w in range(NW):
                wbase = w * WROWS
                wrows = min(WROWS, TABLE_ROWS - wbase)
                for k in range(K):
                    git = gipool.tile([128, S // 16], i16, tag="git")
                    nc.sync.dma_start(out=git[:], in_=gb[w, k, :, :])
                    sit = sipool.tile([128, S // 16], i16, tag="sit")
                    nc.sync.dma_start(out=sit[:], in_=sb[w, k, :, :])
                    gc = gcpool.tile([128, S], bf16, tag="gc")
                    for c0 in range(0, S, GCH):
                        nc.gpsimd.dma_gather(
                            out_ap=gc[:, c0:c0 + GCH].rearrange(
                                "p (a n) -> p a n", a=1),
                            in_ap=src_ap[wbase:wbase + wrows, :],
                            idxs_ap=git[:, c0 // 16:(c0 + GCH) // 16],
                            num_idxs=GCH, num_idxs_reg=GCH,
                            elem_size=C, transpose=True,
                            queue_num=(c0 // GCH) % 4)
                    sd = sdpool.tile([128, S], bf16, tag="sd")
                    for g in range(S // 512):
                        pm = pmp.tile([128, 512], f32, tag="pm")
                        nc.tensor.matmul(
                            pm[:], lhsT=w_sb[:, k * C:(k + 1) * C],
                            rhs=gc[:, g * 512:(g + 1) * 512],
                            start=True, stop=True)
                        gt = gtpool.tile([128, 512], bf16, tag="gt")
                        if g % 2 == 0:
                            nc.vector.tensor_copy(out=gt[:], in_=pm[:])
                        else:
                            nc.scalar.activation(out=gt[:], in_=pm[:],
                                                 func=AF.Identity)
                        pt = ptp.tile([128, 512], bf16, tag="pt")
                        for s in range(4):
                            nc.tensor.transpose(
                                out=pt[:, s * 128:(s + 1) * 128],
                                in_=gt[:, s * 128:(s + 1) * 128],
                                identity=ident[:])
                        if g % 2 == 0:
                            nc.scalar.activation(out=sd[:, g * 512:(g + 1) * 512],
                                                 in_=pt[:], func=AF.Identity)
                        else:
                            nc.vector.tensor_copy(out=sd[:, g * 512:(g + 1) * 512],
                                                  in_=pt[:])
                    pend.append((sit, sd, k))
                    if len(pend) > SC_DELAY:
                        flush_one()
            while pend:
                flush_one()

        def stats(slabs, h_sb, st_sum, st_sq):
            for t in range(nsup):
                r0 = t * SUP * 128
                acc = None
                parts = []
                for i in range(NSLAB):
                    rt = rpool.tile([128, SUP * C], bf16, tag=f"rt{i}")
                    nc.sync.dma_start(
                        out=rt[:].rearrange("p (b c) -> p b c", c=C),
                        in_=slabs[i][r0:r0 + SUP * 128, :].rearrange(
                            "(b p) c -> p b c", p=128))
                    parts.append(rt)
                # tree-sum NSLAB parts on vector
                lvl = 0
                while len(parts) > 1:
                    nxt = []
                    for j in range(0, len(parts) - 1, 2):
                        sm = rpool.tile([128, SUP * C], bf16,
                                        tag=f"sm{lvl}_{j}")
                        nc.vector.tensor_tensor(out=sm[:], in0=parts[j][:],
                                                in1=parts[j + 1][:], op=ALU.add)
                        nxt.append(sm)
                    if len(parts) % 2:
                        nxt.append(parts[-1])
                    parts = nxt
                    lvl += 1
                hsum = parts[0]
                pt = ptp.tile([128, 512], bf16, tag="pt")
                for s in range(SUP):
                    nc.tensor.transpose(out=pt[:, s * 128:(s + 1) * 128],
                                        in_=hsum[:, s * C:(s + 1) * C],
                                        identity=ident[:])
                hs = h_sb[:, r0:r0 + SUP * 128]
                nc.scalar.activation(out=hs, in_=pt[:], func=AF.Identity)
                nc.vector.tensor_reduce(out=st_sum[:, t:t + 1], in_=pt[:],
                                        axis=AX.X, op=ALU.add)
                sq = rpool.tile([128, 512], bf16, tag="sq")
                nc.scalar.activation(out=sq[:], in_=pt[:], func=AF.Square,
                                     accum_out=st_sq[:, t:t + 1])

        def bn_coeffs(st_sum, st_sq, gcol, bcol, cin, cout, name):
            ssum = spool.tile([128, 2], f32, tag=f"pk{name}")
            nc.vector.tensor_reduce(out=ssum[:, 0:1], in_=st_sum[:, :nsup],
                                    axis=AX.X, op=ALU.add)
            nc.vector.tensor_reduce(out=ssum[:, 1:2], in_=st_sq[:, :nsup],
                                    axis=AX.X, op=ALU.add)
            nc.sync.dma_start(out=cin[:, :], in_=ssum[:])
            nc.gpsimd.collective_compute(
                "AllReduce", ALU.add, replica_groups=rg,
                ins=[cin.opt()], outs=[cout.opt()])
            g = spool.tile([128, 2], f32, tag=f"gs{name}")
            nc.sync.dma_start(out=g[:], in_=cout[:, :])
            w = spool.tile([128, 6], f32, tag=f"wk{name}")
            mu, ex2, var = w[:, 0:1], w[:, 1:2], w[:, 2:3]
            nc.vector.tensor_scalar_mul(mu, g[:, 0:1], 1.0 / N_GLOB)
            nc.vector.tensor_scalar_mul(ex2, g[:, 1:2], 1.0 / N_GLOB)
            nc.vector.tensor_tensor(out=var, in0=mu, in1=mu, op=ALU.mult)
            nc.vector.tensor_tensor(out=var, in0=ex2, in1=var, op=ALU.subtract)
            nc.vector.tensor_scalar_add(var, var, EPS)
            sd_ = w[:, 3:4]
            nc.scalar.sqrt(out=sd_, in_=var)
            rstd = w[:, 4:5]
            nc.vector.reciprocal(out=rstd, in_=sd_)
            ab = spool.tile([128, 2], f32, tag=f"ab{name}")
            a, b = ab[:, 0:1], ab[:, 1:2]
            nc.vector.tensor_tensor(out=a, in0=rstd, in1=bns[:, gcol:gcol + 1],
                                    op=ALU.mult)
            t = w[:, 5:6]
            nc.vector.tensor_tensor(out=t, in0=mu, in1=a, op=ALU.mult)
            nc.vector.tensor_tensor(out=b, in0=bns[:, bcol:bcol + 1], in1=t,
                                    op=ALU.subtract)
            return a, b

        # ---- conv1 ----
        zero_slabs(slabs1)
        zero_slabs(slabs2)   # early, overlaps conv1
        conv(x_aug, g1b, s1b, w1s, slabs1)
        h1 = hpool.tile([128, r_pad], bf16, tag="h")
        stats(slabs1, h1, st_sum1, st_sq1)
        a1, b1 = bn_coeffs(st_sum1, st_sq1, 0, 1, ccs_in, ccs_out, "1")

        # ---- bn1 + relu -> row-major -> allgather ----
        for t in range(nsup):
            hs = h1[:, t * 512:(t + 1) * 512]
            o = opool.tile([128, 512], bf16, tag="o")
            nc.scalar.activation(out=o[:], in_=hs, func=AF.Relu,
                                 bias=b1, scale=a1)
            pt = ptp.tile([128, 512], bf16, tag="pt")
            for s in range(SUP):
                nc.tensor.transpose(
                    out=pt[:, s * 128:(s + 1) * 128],
                    in_=o[:, s * 128:(s + 1) * 128], identity=ident[:])
            orow = opool.tile([128, 512], bf16, tag="orow")
            nc.vector.tensor_copy(out=orow[:], in_=pt[:])
            for s in range(SUP):
                r0 = t * 512 + s * 128
                nrows = max(0, min(128, R - r0))
                if nrows:
                    nc.sync.dma_start(out=cc_in[r0:r0 + nrows, :],
                                      in_=orow[:nrows, s * 128:s * 128 + 128])
        nc.gpsimd.collective_compute(
            "AllGather", ALU.bypass, replica_groups=rg,
            ins=[cc_in.opt()], outs=[cc_out[0:N_GLOB, :].opt()])

        # ---- conv2 ----
        conv(cc_out[:, :], g2b, s2b, w2s, slabs2)
        h2 = hpool.tile([128, r_pad], bf16, tag="h")
        stats(slabs2, h2, st_sum2, st_sq2)
        a2, b2 = bn_coeffs(st_sum2, st_sq2, 2, 3, ccs2_in, ccs2_out, "2")

        # ---- bn2 + residual + relu -> y (f32 path) ----
        for t in range(nsup):
            r0 = t * 512
            hs = h2[:, r0:r0 + 512]
            o = opool.tile([128, 512], bf16, tag="o")
            nc.scalar.activation(out=o[:], in_=hs, func=AF.Identity,
                                 bias=b2, scale=a2)
            pt = ptp.tile([128, 512], bf16, tag="pt")
            for s in range(SUP):
                nc.tensor.transpose(
                    out=pt[:, s * 128:(s + 1) * 128],
                    in_=o[:, s * 128:(s + 1) * 128], identity=ident[:])
            rrow = opool.tile([128, 512], f32, tag="rrow")
            nc.vector.tensor_copy(out=rrow[:], in_=pt[:])
            xr = opool.tile([128, 512], f32, tag="xr")
            nc.sync.dma_start(
                out=xr[:].rearrange("p (s c) -> p s c", c=C),
                in_=xres[r0:r0 + 512, :].rearrange("(s p) c -> p s c", p=128))
            nc.vector.tensor_tensor(out=rrow[:], in0=rrow[:], in1=xr[:],
                                    op=ALU.add)
            yt = opool.tile([128, 512], f32, tag="yt")
            nc.scalar.activation(out=yt[:], in_=rrow[:], func=AF.Relu)
            nc.sync.dma_start(
                out=y[r0:r0 + 512, :].rearrange("(s p) c -> p s c", p=128),
                in_=yt[:].rearrange("p (s c) -> p s c", c=C))

    nc.compile()
    return nc


def _wrap16(vals, s):
    """[s] -> [128, s//16] int16: idx i at partition i%16 (replicated x8), col i//16."""
    plane = np.zeros((16, s // 16), np.int16)
    plane[np.arange(len(vals)) % 16, np.arange(len(vals)) // 16] = vals
    return np.tile(plane, (8, 1))


def _prep_conv_idx(kmap, mask, core):
    """Gather/scatter wrap16 idx planes for one conv, one core.

    Returns (gb [NW,K,128,S/16] window-local gather rows,
             sb [NW,K,128,S/16] scatter voxel ids; pads: gather->0,
             scatter->R_PAD sacrificial row)."""
    base = core * R
    eff = kmap[:, base:base + R].astype(np.int64)      # [K, R]
    live = mask[:, base:base + R] != 0
    gb = np.zeros((NW, K, 128, S // 16), np.int16)
    sb = np.full((NW, K, 128, S // 16), R_PAD, np.int16)
    for k in range(K):
        rows = eff[k]
        lv = live[k]
        wins = rows // WROWS
        for w in range(NW):
            m = lv & (wins == w)
            v = np.nonzero(m)[0]
            loc = (rows[v] - w * WROWS).astype(np.int16)
            assert len(v) <= S, f"count {len(v)} > {S} (w={w}, k={k})"
            gplane = np.zeros(S, np.int16)
            gplane[:len(v)] = loc
            splane = np.full(S, R_PAD, np.int16)
            splane[:len(v)] = v.astype(np.int16)
            gb[w, k] = _wrap16(gplane, S)
            sb[w, k] = _wrap16(splane, S)
    return gb, sb


def kernel(x, W1, gamma1, beta1, W2, gamma2, beta2, kmap1, kmap2, mask1, mask2):
    import ml_dtypes
    from concourse import bass_utils
    global LAST_RESULTS

    bf16 = ml_dtypes.bfloat16
    x = np.asarray(x, np.float32)
    x_aug = np.zeros((TABLE_ROWS, C), np.float32)
    x_aug[:N_GLOB] = x
    x_aug_b = x_aug.astype(bf16)

    kmap1 = np.asarray(kmap1)
    kmap2 = np.asarray(kmap2)
    mask1 = np.asarray(mask1)
    mask2 = np.asarray(mask2)
    bnt = np.stack([np.asarray(gamma1, np.float32), np.asarray(beta1, np.float32),
                    np.asarray(gamma2, np.float32), np.asarray(beta2, np.float32)],
                   axis=1)
    w1 = np.ascontiguousarray(np.asarray(W1, np.float32)).astype(bf16)
    w2 = np.ascontiguousarray(np.asarray(W2, np.float32)).astype(bf16)

    if "v4" not in _NC_CACHE:
        _NC_CACHE["v4"] = _build()
    nc = _NC_CACHE["v4"]

    in_maps = []
    for c in range(N_CORES):
        g1 = _prep_conv_idx(kmap1, mask1, c)
        g2 = _prep_conv_idx(kmap2, mask2, c)
        base = c * R
        in_maps.append({
            "x_aug": x_aug_b,
            "g1b": g1[0], "s1b": g1[1],
            "g2b": g2[0], "s2b": g2[1],
            "w1": w1,
            "w2": w2,
            "bnt": bnt,
            "xres": np.ascontiguousarray(x_aug[base:base + R_PAD]),
        })

    kwargs = {}
    if _TRACE:
        kwargs = dict(trace=True, tmpdir=_TMPDIR)
    res = bass_utils.run_bass_kernel_spmd(
        nc, in_maps, core_ids=list(range(N_CORES)), **kwargs)
    LAST_RESULTS = res
    out = np.concatenate([res.results[c]["y"][:R] for c in range(N_CORES)], axis=0)
    return np.ascontiguousarray(out, dtype=np.float32)



# revision 21
# speedup vs baseline: 1.5579x; 1.5579x over previous
"""Trainium2 Bass kernel for nn_BasicBlock — windowed dma_gather + scatter-add design.

Computation (reference):
    h1 = sum_k mask1[k,n] * x[kmap1[k,n]] @ W1[k]
    o1 = relu(bn(h1))
    h2 = sum_k mask2[k,n] * o1[kmap2[k,n]] @ W2[k]
    out = relu(bn(h2) + x)

Voxels sharded 8 ways (25000/core); the feature table (x for conv1, the
all-gathered normalized o1 for conv2) is replicated in DRAM as bf16.

The old per-128-row indirect-DMA gather costs ~1.4us of Pool-sequencer time
per call (measured), an ~15ms floor for 1.35M rows. Instead, each conv is
reorganized around InstDMAGatherAnt/InstDMAScatterAddAnt, whose int16 index
limit is met by splitting the 200256-row table into 7 windows of 32768 rows:

  for each window w, k-triple: one dma_gather(transpose=True) fetches ALL
  live slots (mask=1) whose source row falls in w — compact, int16
  window-local indices, output already transposed [ci, slots];
  per k: W_k^T @ compact -> PSUM; PE-transposes back to row-major;
  one dma_scatter_add adds the rows into h-slab[k%4] at their voxel ids
  (int16, < 25088). Masked slots are never gathered (halves traffic).

h = sum of 4 slabs, computed during a streaming stats pass (transposed into
SBUF); BN stats all-reduced; bn1+relu rows all-gathered (bf16) for conv2;
bn2 + f32 residual + relu written per-core and concatenated on the host.
Index lists are padded to static shapes (gather pad -> window row 0,
scatter pad -> -1 which the ucode ignores), so the program is input-shape
independent; counts are asserted on the host.
"""
import math
from contextlib import ExitStack

import numpy as np

N_GLOB = 200000
C = 128
K = 27
N_CORES = 8
EPS = 1e-5

R = N_GLOB // N_CORES          # 25000 valid rows per core
TILES = math.ceil(R / 128)     # 196
R_PAD = TILES * 128            # 25088
SUP = 4
NSUP = math.ceil(TILES / SUP)  # 49
TABLE_ROWS = ((N_GLOB + 128 + 127) // 128) * 128 + 64  # 200256
ZROW = N_GLOB

WROWS = 28672                  # gather window rows (int16-addressable)
NW = 7                         # uniform windows (last covers 28224 table rows)
S = 2048                       # padded slots per (w, k); gathers issued in 512-chunks
GCH = 512                      # max idx per dma_gather/scatter call (hw ring limit)
NSLAB = 8                      # h accumulation slabs (k % NSLAB)
SC_DELAY = 4                   # scatter issued this many (w,k) iters late
SC_CH = 1024                   # idxs per scatter call (<=1024; 2048 drops data)

_TRACE = False
_TMPDIR = None
LAST_RESULTS = None

_NC_CACHE = {}


def _build():
    from concourse import bass, bacc, tile, mybir

    f32 = mybir.dt.float32
    bf16 = mybir.dt.bfloat16
    i16 = mybir.dt.int16
    AF = mybir.ActivationFunctionType
    ALU = mybir.AluOpType
    AX = mybir.AxisListType

    r_pad = R_PAD
    nsup = NSUP
    rg = [list(range(N_CORES))]

    nc = bacc.Bacc("TRN2", target_bir_lowering=False, debug=False,
                   num_devices=N_CORES, num_swdge_queues=4)

    x_aug = nc.dram_tensor("x_aug", [TABLE_ROWS, C], bf16, kind="ExternalInput").ap()
    # idx planes: [w, k, 128, S/16] (wrap16 layout)
    g1b = nc.dram_tensor("g1b", [NW, K, 128, S // 16], i16, kind="ExternalInput").ap()
    s1b = nc.dram_tensor("s1b", [NW, K, 128, S // 16], i16, kind="ExternalInput").ap()
    g2b = nc.dram_tensor("g2b", [NW, K, 128, S // 16], i16, kind="ExternalInput").ap()
    s2b = nc.dram_tensor("s2b", [NW, K, 128, S // 16], i16, kind="ExternalInput").ap()
    w1d = nc.dram_tensor("w1", [K, C, C], bf16, kind="ExternalInput").ap()
    w2d = nc.dram_tensor("w2", [K, C, C], bf16, kind="ExternalInput").ap()
    bnt = nc.dram_tensor("bnt", [C, 4], f32, kind="ExternalInput").ap()
    xres = nc.dram_tensor("xres", [r_pad, C], f32, kind="ExternalInput").ap()
    y = nc.dram_tensor("y", [r_pad, C], f32, kind="ExternalOutput").ap()

    with tile.TileContext(nc) as tc, ExitStack() as ctx:
        dram = ctx.enter_context(tc.tile_pool(name="dram", bufs=1, space="DRAM"))
        cc_in = dram.tile([R, C], bf16, tag="cc_in")
        cc_out = dram.tile([TABLE_ROWS, C], bf16, tag="cc_out")
        ccs_in = dram.tile([C, 2], f32, tag="ccs_in")
        ccs_out = dram.tile([C, 2], f32, tag="ccs_out")
        ccs2_in = dram.tile([C, 2], f32, tag="ccs2_in")
        ccs2_out = dram.tile([C, 2], f32, tag="ccs2_out")
        slabs1 = [dram.tile([r_pad + 128, C], bf16, tag=f"hs1_{i}", name=f"hs1_{i}")
                  for i in range(NSLAB)]
        slabs2 = [dram.tile([r_pad + 128, C], bf16, tag=f"hs2_{i}", name=f"hs2_{i}")
                  for i in range(NSLAB)]

        wpool = ctx.enter_context(tc.tile_pool(name="wpool", bufs=1))
        hpool = ctx.enter_context(tc.tile_pool(name="hpool", bufs=1))
        spool = ctx.enter_context(tc.tile_pool(name="spool", bufs=1))
        gipool = ctx.enter_context(tc.tile_pool(name="gipool", bufs=12))
        sipool = ctx.enter_context(tc.tile_pool(name="sipool", bufs=12))
        gcpool = ctx.enter_context(tc.tile_pool(name="gcpool", bufs=10))
        sdpool = ctx.enter_context(tc.tile_pool(name="sdpool", bufs=10))
        gtpool = ctx.enter_context(tc.tile_pool(name="gtpool", bufs=6))
        rpool = ctx.enter_context(tc.tile_pool(name="rpool", bufs=2))
        opool = ctx.enter_context(tc.tile_pool(name="opool", bufs=2))
        pmp = ctx.enter_context(tc.tile_pool(name="pmp", bufs=3, space="PSUM"))
        ptp = ctx.enter_context(tc.tile_pool(name="ptp", bufs=4, space="PSUM"))

        # --- resident constants ---
        from concourse.masks import make_identity
        identf = wpool.tile([128, 128], f32, tag="identf")
        make_identity(nc, identf[:])
        ident = wpool.tile([128, 128], bf16, tag="ident")
        nc.vector.tensor_copy(out=ident[:], in_=identf[:])

        w1s = wpool.tile([128, K * C], bf16, tag="w1")
        w2s = wpool.tile([128, K * C], bf16, tag="w2")
        nc.sync.dma_start(out=w1s[:].rearrange("p (k c) -> p k c", c=C),
                          in_=w1d.rearrange("k ci co -> ci k co"))
        nc.sync.dma_start(out=w2s[:].rearrange("p (k c) -> p k c", c=C),
                          in_=w2d.rearrange("k ci co -> ci k co"))
        bns = wpool.tile([128, 4], f32, tag="bns")
        nc.sync.dma_start(out=bns[:], in_=bnt[:, :])

        zt = wpool.tile([128, 8 * C], bf16, tag="zt")
        nc.vector.memset(zt[:], 0.0)

        def zero_slabs(slabs):
            for sl in slabs:
                off = 0
                while off < r_pad:
                    n = min(1024, r_pad - off)
                    nc.sync.dma_start(
                        out=sl[off:off + n, :].rearrange("(b p) c -> p b c", p=128),
                        in_=zt[:].rearrange("p (b c) -> p b c", c=C)[:, :n // 128, :])
                    off += n

        # zero the tail rows of cc_out (mask-skipped rows never read; keep clean)
        off = N_GLOB
        while off < TABLE_ROWS:
            n = min(1024, TABLE_ROWS - off)
            nc.sync.dma_start(
                out=cc_out[off:off + n, :].rearrange("(b p) c -> p b c", p=128),
                in_=zt[:].rearrange("p (b c) -> p b c", c=C)[:, :n // 128, :])
            off += n

        # Pin num_idxs values to dedicated Pool registers, allocated once.
        # Passing plain ints re-MOVEs a shared scratch register per call;
        # with mixed 512/1024 sizes a queued call can read the register
        # after a later call's MOVE flipped it (observed: dropped rows).
        r_gch = nc.gpsimd.to_reg(GCH)
        r_sch = nc.gpsimd.to_reg(SC_CH) if SC_CH != GCH else r_gch

        st_sum1 = spool.tile([128, nsup], f32, tag="st_sum1")
        st_sq1 = spool.tile([128, nsup], f32, tag="st_sq1")
        st_sum2 = spool.tile([128, nsup], f32, tag="st_sum2")
        st_sq2 = spool.tile([128, nsup], f32, tag="st_sq2")

        def conv(src_ap, gb, sb, w_sb, slabs):
            # Software-pipelined: the scatter for iteration i is issued D
            # iterations later, so the in-order Pool queue never stalls on
            # the gather->matmul->transpose->copy chain of the current
            # iteration. One 2048-idx scatter per (w,k) (measured 2.7ns/row,
            # same per-row rate as 512-idx calls at 1/4 the Pool calls).
            pend = []

            def flush_one():
                sit_, sd_, k_ = pend.pop(0)
                nch = S // SC_CH
                for g in range(nch):
                    nc.gpsimd.dma_scatter_add(
                        out_ap=slabs[k_ % NSLAB][:, :],
                        in_ap=sd_[:, g * SC_CH:(g + 1) * SC_CH].rearrange(
                            "p (b c) -> p b c", c=C),
                        idxs_ap=sit_[:, g * SC_CH // 16:(g + 1) * SC_CH // 16],
                        num_idxs=SC_CH, num_idxs_reg=r_sch,
                        elem_size=C, queue_num=k_ % 4)

            for w in range(NW):
                wbase = w * WROWS
                wrows = min(WROWS, TABLE_ROWS - wbase)
                for k in range(K):
                    git = gipool.tile([128, S // 16], i16, tag="git")
                    nc.sync.dma_start(out=git[:], in_=gb[w, k, :, :])
                    sit = sipool.tile([128, S // 16], i16, tag="sit")
                    nc.sync.dma_start(out=sit[:], in_=sb[w, k, :, :])
                    gc = gcpool.tile([128, S], bf16, tag="gc")
                    for c0 in range(0, S, GCH):
                        nc.gpsimd.dma_gather(
                            out_ap=gc[:, c0:c0 + GCH].rearrange(
                                "p (a n) -> p a n", a=1),
                            in_ap=src_ap[wbase:wbase + wrows, :],
                            idxs_ap=git[:, c0 // 16:(c0 + GCH) // 16],
                            num_idxs=GCH, num_idxs_reg=r_gch,
                            elem_size=C, transpose=True,
                            queue_num=(c0 // GCH) % 4)
                    sd = sdpool.tile([128, S], bf16, tag="sd")
                    for g in range(S // 512):
                        pm = pmp.tile([128, 512], f32, tag="pm")
                        nc.tensor.matmul(
                            pm[:], lhsT=w_sb[:, k * C:(k + 1) * C],
                            rhs=gc[:, g * 512:(g + 1) * 512],
                            start=True, stop=True)
                        gt = gtpool.tile([128, 512], bf16, tag="gt")
                        if g % 2 == 0:
                            nc.vector.tensor_copy(out=gt[:], in_=pm[:])
                        else:
                            nc.scalar.activation(out=gt[:], in_=pm[:],
                                                 func=AF.Identity)
                        pt = ptp.tile([128, 512], bf16, tag="pt")
                        for s in range(4):
                            nc.tensor.transpose(
                                out=pt[:, s * 128:(s + 1) * 128],
                                in_=gt[:, s * 128:(s + 1) * 128],
                                identity=ident[:])
                        if g % 2 == 0:
                            nc.scalar.activation(out=sd[:, g * 512:(g + 1) * 512],
                                                 in_=pt[:], func=AF.Identity)
                        else:
                            nc.vector.tensor_copy(out=sd[:, g * 512:(g + 1) * 512],
                                                  in_=pt[:])
                    pend.append((sit, sd, k))
                    if len(pend) > SC_DELAY:
                        flush_one()
            while pend:
                flush_one()

        def stats(slabs, h_sb, st_sum, st_sq):
            for t in range(nsup):
                r0 = t * SUP * 128
                acc = None
                parts = []
                for i in range(NSLAB):
                    rt = rpool.tile([128, SUP * C], bf16, tag=f"rt{i}")
                    nc.sync.dma_start(
                        out=rt[:].rearrange("p (b c) -> p b c", c=C),
                        in_=slabs[i][r0:r0 + SUP * 128, :].rearrange(
                            "(b p) c -> p b c", p=128))
                    parts.append(rt)
                # tree-sum NSLAB parts on vector
                lvl = 0
                while len(parts) > 1:
                    nxt = []
                    for j in range(0, len(parts) - 1, 2):
                        sm = rpool.tile([128, SUP * C], bf16,
                                        tag=f"sm{lvl}_{j}")
                        nc.vector.tensor_tensor(out=sm[:], in0=parts[j][:],
                                                in1=parts[j + 1][:], op=ALU.add)
                        nxt.append(sm)
                    if len(parts) % 2:
                        nxt.append(parts[-1])
                    parts = nxt
                    lvl += 1
                hsum = parts[0]
                pt = ptp.tile([128, 512], bf16, tag="pt")
                for s in range(SUP):
                    nc.tensor.transpose(out=pt[:, s * 128:(s + 1) * 128],
                                        in_=hsum[:, s * C:(s + 1) * C],
                                        identity=ident[:])
                hs = h_sb[:, r0:r0 + SUP * 128]
                nc.scalar.activation(out=hs, in_=pt[:], func=AF.Identity)
                nc.vector.tensor_reduce(out=st_sum[:, t:t + 1], in_=pt[:],
                                        axis=AX.X, op=ALU.add)
                sq = rpool.tile([128, 512], bf16, tag="sq")
                nc.scalar.activation(out=sq[:], in_=pt[:], func=AF.Square,
                                     accum_out=st_sq[:, t:t + 1])

        def bn_coeffs(st_sum, st_sq, gcol, bcol, cin, cout, name):
            ssum = spool.tile([128, 2], f32, tag=f"pk{name}")
            nc.vector.tensor_reduce(out=ssum[:, 0:1], in_=st_sum[:, :nsup],
                                    axis=AX.X, op=ALU.add)
            nc.vector.tensor_reduce(out=ssum[:, 1:2], in_=st_sq[:, :nsup],
                                    axis=AX.X, op=ALU.add)
            nc.sync.dma_start(out=cin[:, :], in_=ssum[:])
            nc.gpsimd.collective_compute(
                "AllReduce", ALU.add, replica_groups=rg,
                ins=[cin.opt()], outs=[cout.opt()])
            g = spool.tile([128, 2], f32, tag=f"gs{name}")
            nc.sync.dma_start(out=g[:], in_=cout[:, :])
            w = spool.tile([128, 6], f32, tag=f"wk{name}")
            mu, ex2, var = w[:, 0:1], w[:, 1:2], w[:, 2:3]
            nc.vector.tensor_scalar_mul(mu, g[:, 0:1], 1.0 / N_GLOB)
            nc.vector.tensor_scalar_mul(ex2, g[:, 1:2], 1.0 / N_GLOB)
            nc.vector.tensor_tensor(out=var, in0=mu, in1=mu, op=ALU.mult)
            nc.vector.tensor_tensor(out=var, in0=ex2, in1=var, op=ALU.subtract)
            nc.vector.tensor_scalar_add(var, var, EPS)
            sd_ = w[:, 3:4]
            nc.scalar.sqrt(out=sd_, in_=var)
            rstd = w[:, 4:5]
            nc.vector.reciprocal(out=rstd, in_=sd_)
            ab = spool.tile([128, 2], f32, tag=f"ab{name}")
            a, b = ab[:, 0:1], ab[:, 1:2]
            nc.vector.tensor_tensor(out=a, in0=rstd, in1=bns[:, gcol:gcol + 1],
                                    op=ALU.mult)
            t = w[:, 5:6]
            nc.vector.tensor_tensor(out=t, in0=mu, in1=a, op=ALU.mult)
            nc.vector.tensor_tensor(out=b, in0=bns[:, bcol:bcol + 1], in1=t,
                                    op=ALU.subtract)
            return a, b

        # ---- conv1 ----
        zero_slabs(slabs1)
        zero_slabs(slabs2)   # early, overlaps conv1
        conv(x_aug, g1b, s1b, w1s, slabs1)
        h1 = hpool.tile([128, r_pad], bf16, tag="h")
        stats(slabs1, h1, st_sum1, st_sq1)
        a1, b1 = bn_coeffs(st_sum1, st_sq1, 0, 1, ccs_in, ccs_out, "1")

        # ---- bn1 + relu -> row-major -> allgather ----
        for t in range(nsup):
            hs = h1[:, t * 512:(t + 1) * 512]
            o = opool.tile([128, 512], bf16, tag="o")
            nc.scalar.activation(out=o[:], in_=hs, func=AF.Relu,
                                 bias=b1, scale=a1)
            pt = ptp.tile([128, 512], bf16, tag="pt")
            for s in range(SUP):
                nc.tensor.transpose(
                    out=pt[:, s * 128:(s + 1) * 128],
                    in_=o[:, s * 128:(s + 1) * 128], identity=ident[:])
            orow = opool.tile([128, 512], bf16, tag="orow")
            nc.vector.tensor_copy(out=orow[:], in_=pt[:])
            for s in range(SUP):
                r0 = t * 512 + s * 128
                nrows = max(0, min(128, R - r0))
                if nrows:
                    nc.sync.dma_start(out=cc_in[r0:r0 + nrows, :],
                                      in_=orow[:nrows, s * 128:s * 128 + 128])
        nc.gpsimd.collective_compute(
            "AllGather", ALU.bypass, replica_groups=rg,
            ins=[cc_in.opt()], outs=[cc_out[0:N_GLOB, :].opt()])

        # ---- conv2 ----
        conv(cc_out[:, :], g2b, s2b, w2s, slabs2)
        h2 = hpool.tile([128, r_pad], bf16, tag="h")
        stats(slabs2, h2, st_sum2, st_sq2)
        a2, b2 = bn_coeffs(st_sum2, st_sq2, 2, 3, ccs2_in, ccs2_out, "2")

        # ---- bn2 + residual + relu -> y (f32 path) ----
        for t in range(nsup):
            r0 = t * 512
            hs = h2[:, r0:r0 + 512]
            o = opool.tile([128, 512], bf16, tag="o")
            nc.scalar.activation(out=o[:], in_=hs, func=AF.Identity,
                                 bias=b2, scale=a2)
            pt = ptp.tile([128, 512], bf16, tag="pt")
            for s in range(SUP):
                nc.tensor.transpose(
                    out=pt[:, s * 128:(s + 1) * 128],
                    in_=o[:, s * 128:(s + 1) * 128], identity=ident[:])
            rrow = opool.tile([128, 512], f32, tag="rrow")
            nc.vector.tensor_copy(out=rrow[:], in_=pt[:])
            xr = opool.tile([128, 512], f32, tag="xr")
            nc.sync.dma_start(
                out=xr[:].rearrange("p (s c) -> p s c", c=C),
                in_=xres[r0:r0 + 512, :].rearrange("(s p) c -> p s c", p=128))
            nc.vector.tensor_tensor(out=rrow[:], in0=rrow[:], in1=xr[:],
                                    op=ALU.add)
            yt = opool.tile([128, 512], f32, tag="yt")
            nc.scalar.activation(out=yt[:], in_=rrow[:], func=AF.Relu)
            nc.sync.dma_start(
                out=y[r0:r0 + 512, :].rearrange("(s p) c -> p s c", p=128),
                in_=yt[:].rearrange("p (s c) -> p s c", c=C))

    nc.compile()
    return nc


def _wrap16(vals, s):
    """[s] -> [128, s//16] int16: idx i at partition i%16 (replicated x8), col i//16."""
    plane = np.zeros((16, s // 16), np.int16)
    plane[np.arange(len(vals)) % 16, np.arange(len(vals)) // 16] = vals
    return np.tile(plane, (8, 1))


def _prep_conv_idx(kmap, mask, core):
    """Gather/scatter wrap16 idx planes for one conv, one core.

    Returns (gb [NW,K,128,S/16] window-local gather rows,
             sb [NW,K,128,S/16] scatter voxel ids; pads: gather->0,
             scatter->R_PAD sacrificial row)."""
    base = core * R
    eff = kmap[:, base:base + R].astype(np.int64)      # [K, R]
    live = mask[:, base:base + R] != 0
    gb = np.zeros((NW, K, 128, S // 16), np.int16)
    sb = np.full((NW, K, 128, S // 16), R_PAD, np.int16)
    for k in range(K):
        rows = eff[k]
        lv = live[k]
        wins = rows // WROWS
        for w in range(NW):
            m = lv & (wins == w)
            v = np.nonzero(m)[0]
            loc = (rows[v] - w * WROWS).astype(np.int16)
            assert len(v) <= S, f"count {len(v)} > {S} (w={w}, k={k})"
            gplane = np.zeros(S, np.int16)
            gplane[:len(v)] = loc
            splane = np.full(S, R_PAD, np.int16)
            splane[:len(v)] = v.astype(np.int16)
            gb[w, k] = _wrap16(gplane, S)
            sb[w, k] = _wrap16(splane, S)
    return gb, sb


def kernel(x, W1, gamma1, beta1, W2, gamma2, beta2, kmap1, kmap2, mask1, mask2):
    import ml_dtypes
    from concourse import bass_utils
    global LAST_RESULTS

    bf16 = ml_dtypes.bfloat16
    x = np.asarray(x, np.float32)
    x_aug = np.zeros((TABLE_ROWS, C), np.float32)
    x_aug[:N_GLOB] = x
    x_aug_b = x_aug.astype(bf16)

    kmap1 = np.asarray(kmap1)
    kmap2 = np.asarray(kmap2)
    mask1 = np.asarray(mask1)
    mask2 = np.asarray(mask2)
    bnt = np.stack([np.asarray(gamma1, np.float32), np.asarray(beta1, np.float32),
                    np.asarray(gamma2, np.float32), np.asarray(beta2, np.float32)],
                   axis=1)
    w1 = np.ascontiguousarray(np.asarray(W1, np.float32)).astype(bf16)
    w2 = np.ascontiguousarray(np.asarray(W2, np.float32)).astype(bf16)

    if "v4" not in _NC_CACHE:
        _NC_CACHE["v4"] = _build()
    nc = _NC_CACHE["v4"]

    in_maps = []
    for c in range(N_CORES):
        g1 = _prep_conv_idx(kmap1, mask1, c)
        g2 = _prep_conv_idx(kmap2, mask2, c)
        base = c * R
        in_maps.append({
            "x_aug": x_aug_b,
            "g1b": g1[0], "s1b": g1[1],
            "g2b": g2[0], "s2b": g2[1],
            "w1": w1,
            "w2": w2,
            "bnt": bnt,
            "xres": np.ascontiguousarray(x_aug[base:base + R_PAD]),
        })

    kwargs = {}
    if _TRACE:
        kwargs = dict(trace=True, tmpdir=_TMPDIR)
    res = bass_utils.run_bass_kernel_spmd(
        nc, in_maps, core_ids=list(range(N_CORES)), **kwargs)
    LAST_RESULTS = res
    out = np.concatenate([res.results[c]["y"][:R] for c in range(N_CORES)], axis=0)
    return np.ascontiguousarray(out, dtype=np.float32)

